# revision 1
# baseline (speedup 1.0000x reference)
"""Trainium2 Bass kernel for nn_Attention (dense transformer MHA forward).

Shapes: x [4096, 1024], 16 heads x head_dim 64, full softmax attention.

Sharding (8 cores, tensor-parallel over heads): each core owns 2 heads.
  - column-parallel qkv: core computes q,k,v for its 2 heads only
  - local attention for 2 heads
  - row-parallel proj: core computes a partial [4096, 1024] output
  - "all-reduce" = host-side sum of the 8 partials (+ b_proj once)

Device-kernel layout choices (per core, heads h0/h1):
  - everything is computed transposed: qkv^T [rows, seq] so that
    S^T = K^T.T-matmul works with seq-k on PSUM partitions and seq-q as the
    moving free dim; the softmax denominator is produced by the PE itself via
    a ones-column folded into the stationary V operand of the P@V matmul.
  - exp() runs on the Scalar engine straight out of PSUM (fused eviction),
    with the 1/sqrt(head_dim) folded into the activation's free scale.
    No max-subtraction: scores are ~N(0,1), exp is safe in fp32.
"""

import numpy as np

SEQ = 4096
DIM = 1024
HEADS = 16
HD = 64
NCORES = 8
QCH = 512          # q-chunk (matmul moving free dim)
KCH = 128          # k-chunk (contraction tile)
NQ = SEQ // QCH    # 8
NK = SEQ // KCH    # 32
NDC = DIM // 128   # 8 contraction chunks for the qkv projection

_COMPILED = {}


def _build_nc(loop_n=None, exp_bf16=False, ablate=None, cdt_name='float32r'):
    import concourse.bass as bass
    import concourse.bacc as bacc
    from concourse import mybir, tile

    f32 = mybir.dt.float32
    f32r = getattr(mybir.dt, cdt_name)  # compute dtype for all matmul operands
    bf16 = mybir.dt.bfloat16
    edt = bf16 if exp_bf16 else f32r
    nc = bacc.Bacc("TRN2", target_bir_lowering=False, debug=False)

    xT_d = nc.dram_tensor("xT", [NQ, 128, NDC, QCH], f32r, kind="ExternalInput")
    wqkvT_d = nc.dram_tensor("wqkvT", [128, NDC, 384], f32r, kind="ExternalInput")
    bq_d = nc.dram_tensor("bq", [128, 3], f32, kind="ExternalInput")
    wprojT_d = nc.dram_tensor("wprojT", [128, DIM], f32r, kind="ExternalInput")
    sel_d = nc.dram_tensor("sel", [128, 128], f32r, kind="ExternalInput")
    ident_d = nc.dram_tensor("ident", [128, 64], f32r, kind="ExternalInput")
    vfill_d = nc.dram_tensor("vfill", [128, NK, 65], edt, kind="ExternalInput")
    zfill_d = nc.dram_tensor("zfill", [128, QCH], f32r, kind="ExternalInput")
    efill_d = None
    if ablate == "noexp":
        efill_d = nc.dram_tensor("efill", [128, 2 * QCH], edt, kind="ExternalInput")
    y_d = nc.dram_tensor("y", [SEQ, DIM], f32, kind="ExternalOutput")

    EXP = mybir.ActivationFunctionType.Exp

    with tile.TileContext(nc) as tc, nc.allow_low_precision(
        reason="float32r (11-bit mantissa) matmul inputs, fp32 PSUM accumulate"
    ):
        with (
            tc.tile_pool(name="const", bufs=1) as const,
            tc.tile_pool(name="xpool", bufs=2) as xpool,
            tc.tile_pool(name="big", bufs=1) as big,
            tc.tile_pool(name="epool", bufs=3) as epool,
            tc.tile_pool(name="opool", bufs=2) as opool,
            tc.tile_pool(name="ypool", bufs=3) as ypool,
            tc.tile_pool(name="spsum", bufs=2, space="PSUM") as spsum,
            tc.tile_pool(name="opsum", bufs=1, space="PSUM") as opsum,
            tc.tile_pool(name="mpsum", bufs=2, space="PSUM") as mpsum,
        ):
            # ---- constants ----
            wq = const.tile([128, NDC, 384], f32r)
            nc.sync.dma_start(out=wq, in_=wqkvT_d.ap())
            wp = const.tile([128, DIM], f32r)
            nc.sync.dma_start(out=wp, in_=wprojT_d.ap())
            bq = const.tile([128, 3], f32)
            nc.sync.dma_start(out=bq, in_=bq_d.ap())
            sel = const.tile([128, 128], f32r)
            nc.sync.dma_start(out=sel, in_=sel_d.ap())
            idn = const.tile([128, 64], f32r)
            nc.sync.dma_start(out=idn, in_=ident_d.ap())
            e_const = None
            if ablate == "noexp":
                e_const = const.tile([128, 2 * QCH], edt, name="e_const")
                nc.sync.dma_start(out=e_const, in_=efill_d.ap())

            # ---- persistent SBUF state ----
            KT = big.tile([128, SEQ], f32r)   # rows 0:64 K^T h0, 64:128 K^T h1
            VT = big.tile([128, SEQ], f32r)
            QT = big.tile([128, SEQ], f32r)
            # per k-chunk stationary for P@V:
            #   cols 0:64 V_h0 | 64 ones | then h1 slab (65:193):
            #   local [0:32] zeros | [32] ones | [33:64] zeros | [64:128] V_h1
            #   so h1's Z lands on PSUM partition 32 (32-aligned APs only)
            vall = big.tile([128, NK, 193], edt)
            zsb = big.tile([128, QCH], f32r)  # softmax-recip staging rows 63/64

            # memset can't write f32r; DMA the static ones/zeros pattern in
            nc.sync.dma_start(out=zsb, in_=zfill_d.ap())
            nc.sync.dma_start(out=vall[:, :, 64:129], in_=vfill_d.ap())

            import contextlib
            loop_cm = (
                tc.For_i(0, loop_n, 1, hint_engines=(
                    mybir.EngineType.PE, mybir.EngineType.DVE,
                    mybir.EngineType.Activation, mybir.EngineType.SP,
                    mybir.EngineType.Pool,
                ))
                if loop_n else contextlib.nullcontext()
            )
            with loop_cm:
                # ---- qkv projection: qkv^T[row, s] for 384 rows = [K0 K1 V0 V1 Q0 Q1]
                xTr = xT_d.ap()  # host-tiled [sc, 128, dc, q] for contiguous DMA
                dests = [KT, VT, QT]

                def emit_qkv_sc(sc):
                    xs = xpool.tile([128, NDC, QCH], f32r, tag="xs", name="xs")
                    nc.sync.dma_start(out=xs[:, 0:2, :], in_=xTr[sc, :, 0:2, :])
                    nc.sync.dma_start(out=xs[:, 2:5, :], in_=xTr[sc, :, 2:5, :])
                    nc.sync.dma_start(out=xs[:, 5:8, :], in_=xTr[sc, :, 5:8, :])
                    for m in range(3):
                        ps = mpsum.tile([128, QCH], f32, tag="mm", name="ps")
                        for dc in range(NDC):
                            nc.tensor.matmul(
                                ps,
                                lhsT=wq[:, dc, m * 128:(m + 1) * 128],
                                rhs=xs[:, dc, :],
                                start=(dc == 0),
                                stop=(dc == NDC - 1),
                            )
                        nc.vector.tensor_scalar_add(
                            dests[m][:, sc * QCH:(sc + 1) * QCH], ps, bq[:, m:m + 1]
                        )
                    # V^T -> V transposes for this s-chunk's 4 k-chunks
                    for kc in range(4 * sc, 4 * sc + 4):
                        for h in range(2):
                            tp = mpsum.tile([128, 64], f32r, tag="mm", name="tp")
                            nc.tensor.transpose(
                                tp, VT[64 * h:64 * h + 64, kc * 128:(kc + 1) * 128],
                                idn[64 * h:64 * h + 64, :]
                            )
                            dst = 0 if h == 0 else 129
                            nc.vector.tensor_copy(vall[:, kc, dst:dst + 64], tp)

                def emit_pv(e, kc, o0, o1):
                    nc.tensor.matmul(
                        o0[0:65, :], lhsT=vall[:, kc, 0:65], rhs=e[:, 0:QCH],
                        start=(kc == 0), stop=(kc == NK - 1),
                    )
                    if ablate != "mm2":
                        nc.tensor.matmul(
                            o1, lhsT=vall[:, kc, 65:193], rhs=e[:, QCH:2 * QCH],
                            start=(kc == 0), stop=(kc == NK - 1),
                        )

                def emit_tail(o0, o1):
                    if ablate == "mm2":
                        o1 = o0  # timing probe only: h1 accumulator unused
                    # softmax denominators: Z0 at o0 row 64, Z1 at o1 row 32
                    nc.vector.reciprocal(zsb[64:65, :], o0[64:65, :])
                    nc.vector.reciprocal(zsb[32:33, :], o1[32:33, :])
                    zb = mpsum.tile([128, QCH], f32, tag="mm", name="zb")
                    nc.tensor.matmul(zb, lhsT=sel, rhs=zsb, start=True, stop=True)
                    zbs = opool.tile([128, QCH], f32, tag="zbs", name="zbs")
                    nc.vector.tensor_copy(zbs, zb)
                    ot = opool.tile([128, QCH], f32r, tag="ot", name="ot")
                    nc.vector.tensor_mul(ot[0:64, :], o0[0:64, :], zbs[0:64, :])
                    nc.vector.tensor_mul(ot[64:128, :], o1[64:128, :], zbs[64:128, :])
                    return ot

                def emit_proj(ot, qc):
                    for ss in range(4):
                        ysb = ypool.tile([128, DIM], f32, tag="y", name="ysb")
                        for oh in range(2):
                            yp = mpsum.tile([128, QCH], f32, tag="mm", name="yp")
                            nc.tensor.matmul(
                                yp,
                                lhsT=ot[:, ss * 128:(ss + 1) * 128],
                                rhs=wp[:, oh * QCH:(oh + 1) * QCH],
                                start=True, stop=True,
                            )
                            nc.vector.tensor_copy(ysb[:, oh * QCH:(oh + 1) * QCH], yp)
                        r0 = qc * QCH + ss * 128
                        nc.sync.dma_start(out=y_d.ap()[r0:r0 + 128, :], in_=ysb)

                # ---- attention, software-pipelined across engines ----
                # Per (qc, kc) iteration this emits, in PE program order:
                #   S-pair(qc,kc) -> [tail(qc-1) @ kc==1] -> PV(prev iter)
                #   -> [proj(qc-1) @ kc==3]
                # so the PE never sits behind the exp of its own iteration, and
                # the normalize/proj of the previous q-chunk hides inside this one.
                pend = {"pv": None, "tail": None, "proj": None}

                def attn_iter(qc, kc, o0, o1):
                    qsl = slice(qc * QCH, (qc + 1) * QCH)
                    ksl = slice(kc * 128, (kc + 1) * 128)
                    s_ps = spsum.tile([128, 2 * QCH], f32, tag="s", name="s_ps")
                    if ablate == "mm2":
                        nc.tensor.matmul(
                            s_ps[:, 0:QCH], lhsT=KT[:, ksl], rhs=QT[:, qsl],
                            start=True, stop=True,
                        )
                    else:
                        nc.tensor.matmul(
                            s_ps[:, 0:QCH], lhsT=KT[0:64, ksl], rhs=QT[0:64, qsl],
                            start=True, stop=True,
                        )
                        nc.tensor.matmul(
                            s_ps[:, QCH:2 * QCH], lhsT=KT[64:128, ksl],
                            rhs=QT[64:128, qsl],
                            start=True, stop=True,
                        )
                    if ablate == "noexp":
                        e = e_const
                    else:
                        e = epool.tile([128, 2 * QCH], edt, tag="e", name="e")
                        nc.scalar.activation(e, s_ps, EXP, scale=1.0 / np.sqrt(HD))
                    if kc == 1 and pend["tail"] is not None:
                        prev_qc, po0, po1 = pend["tail"]
                        pend["proj"] = (emit_tail(po0, po1), prev_qc)
                        pend["tail"] = None
                    if pend["pv"] is not None:
                        emit_pv(*pend["pv"])
                    pend["pv"] = (e, kc, o0, o1)
                    if kc == 3 and pend["proj"] is not None:
                        emit_proj(*pend["proj"])
                        pend["proj"] = None

                # qc 0 rides along with the qkv prologue: each s-chunk of qkv
                # unlocks 4 k-chunks of attention for q-chunk 0
                o0 = opsum.tile([128, QCH], f32, tag="o0", name="o0")
                o1 = opsum.tile([128, QCH], f32, tag="o1", name="o1")
                for sc in range(NQ):
                    emit_qkv_sc(sc)
                    for kc in range(4 * sc, 4 * sc + 4):
                        attn_iter(0, kc, o0, o1)

                for qc in range(1, NQ):
                    po0, po1 = o0, o1
                    o0 = opsum.tile([128, QCH], f32, tag="o0", name="o0")
                    o1 = opsum.tile([128, QCH], f32, tag="o1", name="o1")
                    pend["tail"] = (qc - 1, po0, po1)
                    for kc in range(NK):
                        attn_iter(qc, kc, o0, o1)

                emit_pv(*pend["pv"])
                emit_proj(emit_tail(o0, o1), NQ - 1)

    nc.compile()
    return nc


def _round_fp32r(a):
    """Round to the compute dtype: fp32r (11-bit mantissa), fp16, or bf16."""
    if CDT == "float16":
        return np.ascontiguousarray(a).astype(np.float16)
    if CDT == "bfloat16":
        import ml_dtypes
        return np.ascontiguousarray(a).astype(ml_dtypes.bfloat16)
    b = np.ascontiguousarray(a).view(np.uint32)
    lsb = (b >> np.uint32(12)) & np.uint32(1)
    out = (b + np.uint32(0x7FF) + lsb) & np.uint32(0xFFFFF000)
    return out.view(np.float32)


def _cdt_np(a):
    if CDT == "float16":
        return a.astype(np.float16)
    if CDT == "bfloat16":
        import ml_dtypes
        return a.astype(ml_dtypes.bfloat16)
    return a


def _prep_inputs(x, W_qkv, b_qkv, W_proj):
    """Host-side shard prep. Returns per-core input maps for the SPMD kernel."""
    # [sc, p, dc, q] layout: xt[sc, p, dc, q] = x[sc*512+q, dc*128+p]
    xT = _round_fp32r(np.ascontiguousarray(
        x.reshape(NQ, QCH, NDC, 128).transpose(0, 3, 2, 1)))
    sel = np.zeros((128, 128), dtype=np.float32)
    sel[64, 0:64] = 1.0  # zsb partition 64 (recip Z0) -> bcast rows 0:64
    sel[32, 64:128] = 1.0  # zsb partition 32 (recip Z1) -> bcast rows 64:128
    sel = _cdt_np(sel)
    ident = _cdt_np(np.ascontiguousarray(np.vstack([np.eye(64, dtype=np.float32)] * 2)))
    patt = np.zeros(65, dtype=np.float32)
    patt[0] = 1.0   # vall col 64: ones column for head 0 sums
    patt[33] = 1.0  # vall col 97: ones column for head 1 sums (partition 32)
    vfill = np.ascontiguousarray(np.broadcast_to(patt, (128, NK, 65)))
    if EXP_BF16:
        import ml_dtypes
        vfill = vfill.astype(ml_dtypes.bfloat16)
    else:
        vfill = _cdt_np(vfill)
    zfill = _cdt_np(np.zeros((128, QCH), dtype=np.float32))
    efill = np.ones((128, 2 * QCH), dtype=np.float32)

    in_maps = []
    for c in range(NCORES):
        h0 = 2 * c
        idx = np.concatenate([
            np.arange(DIM + HD * h0, DIM + HD * h0 + 128),          # K rows
            np.arange(2 * DIM + HD * h0, 2 * DIM + HD * h0 + 128),  # V rows
            np.arange(HD * h0, HD * h0 + 128),                      # Q rows
        ])
        w_shard = W_qkv[idx]                                  # [384, 1024]
        # [p, dc, row]: wq[p, dc, r] = w_shard[r, dc*128+p]
        wqkvT = _round_fp32r(np.ascontiguousarray(
            w_shard.T.reshape(NDC, 128, 384).transpose(1, 0, 2)))
        bq = np.ascontiguousarray(b_qkv[idx].reshape(3, 128).T)  # [128, 3]
        wprojT = _round_fp32r(np.ascontiguousarray(W_proj[:, 128 * c:128 * (c + 1)].T))  # [128, 1024]
        in_maps.append({
            "xT": xT,
            "wqkvT": wqkvT,
            "bq": bq,
            "wprojT": wprojT,
            "sel": sel,
            "ident": ident,
            "vfill": vfill,
            "zfill": zfill,
        })
        if ABLATE == "noexp":
            in_maps[-1]["efill"] = efill
    return in_maps


EXP_BF16 = False
ABLATE = None
CDT = "float32r"


def _get_nc(loop_n=None):
    key = ("nc", loop_n, EXP_BF16, ABLATE, CDT)
    if key not in _COMPILED:
        _COMPILED[key] = _build_nc(
            loop_n, exp_bf16=EXP_BF16, ablate=ABLATE, cdt_name=CDT)
    return _COMPILED[key]


def run(x, W_qkv, b_qkv, W_proj, b_proj, trace=False, **trace_kwargs):
    """Run the sharded kernel; returns (y_full, BassKernelResults)."""
    from concourse.bass_utils import run_bass_kernel_spmd

    x = np.asarray(x, dtype=np.float32)
    W_qkv = np.asarray(W_qkv, dtype=np.float32)
    b_qkv = np.asarray(b_qkv, dtype=np.float32)
    W_proj = np.asarray(W_proj, dtype=np.float32)
    b_proj = np.asarray(b_proj, dtype=np.float32)

    nc = _get_nc()
    in_maps = _prep_inputs(x, W_qkv, b_qkv, W_proj)
    res = run_bass_kernel_spmd(
        nc, in_maps, core_ids=list(range(NCORES)), trace=trace, **trace_kwargs
    )
    y = np.zeros((SEQ, DIM), dtype=np.float32)
    for r in res.results:
        y += r["y"]
    y += b_proj
    return y, res


def kernel(x, W_qkv, b_qkv, W_proj, b_proj):
    y, _ = run(x, W_qkv, b_qkv, W_proj, b_proj, trace=False)
    return y



# revision 13
# speedup vs baseline: 6.1776x; 6.1776x over previous
"""Trainium2 Bass kernel for nn_Attention (dense transformer MHA forward).

Shapes: x [4096, 1024], 16 heads x head_dim 64, full softmax attention.

Sharding (8 cores, tensor-parallel over heads): each core owns 2 heads.
  - column-parallel qkv: core computes q,k,v for its 2 heads only
  - local attention for 2 heads
  - row-parallel proj: core computes a partial [4096, 1024] output
  - "all-reduce" = host-side sum of the 8 partials (+ b_proj once)

Device-kernel layout choices (per core, heads h0/h1):
  - everything is computed transposed: qkv^T [rows, seq] so that
    S^T = K^T.T-matmul works with seq-k on PSUM partitions and seq-q as the
    moving free dim; the softmax denominator is produced by the PE itself via
    a ones-column folded into the stationary V operand of the P@V matmul.
  - exp() runs on the Scalar engine straight out of PSUM (fused eviction),
    with the 1/sqrt(head_dim) folded into the activation's free scale.
    No max-subtraction: scores are ~N(0,1), exp is safe in fp32.
"""

import numpy as np

SEQ = 4096
DIM = 1024
HEADS = 16
HD = 64
NCORES = 8
QCH = 512          # q-chunk (matmul moving free dim)
KCH = 128          # k-chunk (contraction tile)
NQ = SEQ // QCH    # 8
NK = SEQ // KCH    # 32
NDC = DIM // 128   # 8 contraction chunks for the qkv projection

_COMPILED = {}


def _build_nc(loop_n=None, exp_bf16=False, ablate=None, cdt_name='float32r'):
    abl = set((ablate or '').split('+')) - {''}
    import concourse.bass as bass
    import concourse.bacc as bacc
    from concourse import mybir, tile

    f32 = mybir.dt.float32
    f32r = getattr(mybir.dt, cdt_name)  # compute dtype for all matmul operands
    bf16 = mybir.dt.bfloat16
    edt = bf16 if exp_bf16 else f32r
    nc = bacc.Bacc("TRN2", target_bir_lowering=False, debug=False)

    xT_d = nc.dram_tensor("xT", [NQ, 128, NDC, QCH], f32r, kind="ExternalInput")
    wqkvT_d = nc.dram_tensor("wqkvT", [128, NDC, 384], f32r, kind="ExternalInput")
    bq_d = nc.dram_tensor("bq", [128, 3], f32, kind="ExternalInput")
    wprojT_d = nc.dram_tensor("wprojT", [128, DIM], f32r, kind="ExternalInput")
    sel_d = nc.dram_tensor("sel", [128, 128], f32r, kind="ExternalInput")
    ident_d = nc.dram_tensor("ident", [128, 64], f32r, kind="ExternalInput")
    vfill_d = nc.dram_tensor("vfill", [128, NK, 65], edt, kind="ExternalInput")
    zfill_d = nc.dram_tensor("zfill", [128, QCH], f32r, kind="ExternalInput")
    efill_d = None
    if "noexp" in abl:
        efill_d = nc.dram_tensor("efill", [128, 2 * QCH], edt, kind="ExternalInput")
    y_d = nc.dram_tensor("y", [SEQ, DIM], f32, kind="ExternalOutput")

    EXP = mybir.ActivationFunctionType.Exp

    with tile.TileContext(nc) as tc, nc.allow_low_precision(
        reason="float32r (11-bit mantissa) matmul inputs, fp32 PSUM accumulate"
    ):
        with (
            tc.tile_pool(name="const", bufs=1) as const,
            tc.tile_pool(name="xpool", bufs=2) as xpool,
            tc.tile_pool(name="big", bufs=1) as big,
            tc.tile_pool(name="epool", bufs=3) as epool,
            tc.tile_pool(name="opool", bufs=2) as opool,
            tc.tile_pool(name="ypool", bufs=3) as ypool,
            tc.tile_pool(name="spsum", bufs=2, space="PSUM") as spsum,
            tc.tile_pool(name="opsum", bufs=1, space="PSUM") as opsum,
            tc.tile_pool(name="mpsum", bufs=2, space="PSUM") as mpsum,
        ):
            # ---- constants ----
            wq = const.tile([128, NDC, 384], f32r)
            nc.sync.dma_start(out=wq, in_=wqkvT_d.ap())
            wp = const.tile([128, DIM], f32r)
            nc.sync.dma_start(out=wp, in_=wprojT_d.ap())
            bq = const.tile([128, 3], f32)
            nc.sync.dma_start(out=bq, in_=bq_d.ap())
            sel = const.tile([128, 128], f32r)
            nc.sync.dma_start(out=sel, in_=sel_d.ap())
            idn = const.tile([128, 64], f32r)
            nc.sync.dma_start(out=idn, in_=ident_d.ap())
            e_const = None
            if "noexp" in abl:
                e_const = const.tile([128, 2 * QCH], edt, name="e_const")
                nc.sync.dma_start(out=e_const, in_=efill_d.ap())

            # ---- persistent SBUF state ----
            KT = big.tile([128, SEQ], f32r)   # rows 0:64 K^T h0, 64:128 K^T h1
            VT = big.tile([128, SEQ], f32r)
            QT = big.tile([128, SEQ], f32r)
            # per k-chunk stationary for P@V:
            #   cols 0:64 V_h0 | 64 ones | then h1 slab (65:193):
            #   local [0:32] zeros | [32] ones | [33:64] zeros | [64:128] V_h1
            #   so h1's Z lands on PSUM partition 32 (32-aligned APs only)
            vall = big.tile([128, NK, 193], edt)
            zsb = big.tile([128, QCH], f32r)  # softmax-recip staging rows 63/64

            # memset can't write f32r; DMA the static ones/zeros pattern in
            nc.sync.dma_start(out=zsb, in_=zfill_d.ap())
            nc.sync.dma_start(out=vall[:, :, 64:129], in_=vfill_d.ap())

            import contextlib
            loop_cm = (
                tc.For_i(0, loop_n, 1, hint_engines=(
                    mybir.EngineType.PE, mybir.EngineType.DVE,
                    mybir.EngineType.Activation, mybir.EngineType.SP,
                    mybir.EngineType.Pool,
                ))
                if loop_n else contextlib.nullcontext()
            )
            with loop_cm:
                # ---- qkv projection: qkv^T[row, s] for 384 rows = [K0 K1 V0 V1 Q0 Q1]
                xTr = xT_d.ap()  # host-tiled [sc, 128, dc, q] for contiguous DMA
                dests = [KT, VT, QT]

                def emit_qkv_sc(sc):
                    xs = xpool.tile([128, NDC, QCH], f32r, tag="xs", name="xs")
                    if "nodma" in abl:
                        # timing probe: 1/8 of input traffic, rest of xs stays garbage
                        nc.sync.dma_start(out=xs[:, 0:1, :], in_=xTr[sc, :, 0:1, :])
                    else:
                        nc.sync.dma_start(out=xs[:, 0:2, :], in_=xTr[sc, :, 0:2, :])
                        nc.sync.dma_start(out=xs[:, 2:5, :], in_=xTr[sc, :, 2:5, :])
                        nc.sync.dma_start(out=xs[:, 5:8, :], in_=xTr[sc, :, 5:8, :])
                    for m in range(3):
                        ps = mpsum.tile([128, QCH], f32, tag="mm", name="ps")
                        for dc in range(NDC):
                            nc.tensor.matmul(
                                ps,
                                lhsT=wq[:, dc, m * 128:(m + 1) * 128],
                                rhs=xs[:, dc, :],
                                start=(dc == 0),
                                stop=(dc == NDC - 1),
                            )
                        nc.vector.tensor_scalar_add(
                            dests[m][:, sc * QCH:(sc + 1) * QCH], ps, bq[:, m:m + 1]
                        )
                    # V^T -> V transposes for this s-chunk's 4 k-chunks
                    for kc in range(4 * sc, 4 * sc + 4):
                        for h in range(2):
                            tp = mpsum.tile([128, 64], f32r, tag="mm", name="tp")
                            nc.tensor.transpose(
                                tp, VT[64 * h:64 * h + 64, kc * 128:(kc + 1) * 128],
                                idn[64 * h:64 * h + 64, :]
                            )
                            dst = 0 if h == 0 else 129
                            nc.vector.tensor_copy(vall[:, kc, dst:dst + 64], tp)

                def emit_pv(e, kc, o0, o1):
                    nc.tensor.matmul(
                        o0[0:65, :], lhsT=vall[:, kc, 0:65], rhs=e[:, 0:QCH],
                        start=(kc == 0), stop=(kc == NK - 1),
                    )
                    if "mm2" not in abl:
                        nc.tensor.matmul(
                            o1, lhsT=vall[:, kc, 65:193], rhs=e[:, QCH:2 * QCH],
                            start=(kc == 0), stop=(kc == NK - 1),
                        )

                def emit_tail(o0, o1):
                    if "mm2" in abl:
                        o1 = o0  # timing probe only: h1 accumulator unused
                    # softmax denominators: Z0 at o0 row 64, Z1 at o1 row 32
                    nc.vector.reciprocal(zsb[64:65, :], o0[64:65, :])
                    nc.vector.reciprocal(zsb[32:33, :], o1[32:33, :])
                    zb = mpsum.tile([128, QCH], f32, tag="mm", name="zb")
                    nc.tensor.matmul(zb, lhsT=sel, rhs=zsb, start=True, stop=True)
                    zbs = opool.tile([128, QCH], f32, tag="zbs", name="zbs")
                    nc.vector.tensor_copy(zbs, zb)
                    ot = opool.tile([128, QCH], f32r, tag="ot", name="ot")
                    nc.vector.tensor_mul(ot[0:64, :], o0[0:64, :], zbs[0:64, :])
                    nc.vector.tensor_mul(ot[64:128, :], o1[64:128, :], zbs[64:128, :])
                    return ot

                def emit_proj(ot, qc):
                    for ss in range(4):
                        ysb = ypool.tile([128, DIM], f32, tag="y", name="ysb")
                        for oh in range(2):
                            yp = mpsum.tile([128, QCH], f32, tag="mm", name="yp")
                            nc.tensor.matmul(
                                yp,
                                lhsT=ot[:, ss * 128:(ss + 1) * 128],
                                rhs=wp[:, oh * QCH:(oh + 1) * QCH],
                                start=True, stop=True,
                            )
                            nc.vector.tensor_copy(ysb[:, oh * QCH:(oh + 1) * QCH], yp)
                        r0 = qc * QCH + ss * 128
                        if "nodma" not in abl:
                            nc.sync.dma_start(out=y_d.ap()[r0:r0 + 128, :], in_=ysb)

                # ---- attention, software-pipelined across engines ----
                # Per (qc, kc) iteration this emits, in PE program order:
                #   S-pair(qc,kc) -> [tail(qc-1) @ kc==1] -> PV(prev iter)
                #   -> [proj(qc-1) @ kc==3]
                # so the PE never sits behind the exp of its own iteration, and
                # the normalize/proj of the previous q-chunk hides inside this one.
                pend = {"pv": None, "tail": None, "proj": None}

                def attn_iter(qc, kc, o0, o1):
                    qsl = slice(qc * QCH, (qc + 1) * QCH)
                    ksl = slice(kc * 128, (kc + 1) * 128)
                    s_ps = spsum.tile([128, 2 * QCH], f32, tag="s", name="s_ps")
                    if "mm2" in abl:
                        nc.tensor.matmul(
                            s_ps[:, 0:QCH], lhsT=KT[:, ksl], rhs=QT[:, qsl],
                            start=True, stop=True,
                        )
                    else:
                        nc.tensor.matmul(
                            s_ps[:, 0:QCH], lhsT=KT[0:64, ksl], rhs=QT[0:64, qsl],
                            start=True, stop=True,
                        )
                        nc.tensor.matmul(
                            s_ps[:, QCH:2 * QCH], lhsT=KT[64:128, ksl],
                            rhs=QT[64:128, qsl],
                            start=True, stop=True,
                        )
                    if "noexp" in abl:
                        e = e_const
                    else:
                        e = epool.tile([128, 2 * QCH], edt, tag="e", name="e")
                        nc.scalar.activation(e, s_ps, EXP, scale=1.0 / np.sqrt(HD))
                    if kc == 1 and pend["tail"] is not None:
                        prev_qc, po0, po1 = pend["tail"]
                        pend["proj"] = (emit_tail(po0, po1), prev_qc)
                        pend["tail"] = None
                    if pend["pv"] is not None:
                        emit_pv(*pend["pv"])
                    pend["pv"] = (e, kc, o0, o1)
                    if kc == 3 and pend["proj"] is not None:
                        emit_proj(*pend["proj"])
                        pend["proj"] = None

                # qc 0 rides along with the qkv prologue: each s-chunk of qkv
                # unlocks 4 k-chunks of attention for q-chunk 0
                o0 = opsum.tile([128, QCH], f32, tag="o0", name="o0")
                o1 = opsum.tile([128, QCH], f32, tag="o1", name="o1")
                for sc in range(NQ):
                    emit_qkv_sc(sc)
                    for kc in range(4 * sc, 4 * sc + 4):
                        attn_iter(0, kc, o0, o1)

                for qc in range(1, NQ):
                    po0, po1 = o0, o1
                    o0 = opsum.tile([128, QCH], f32, tag="o0", name="o0")
                    o1 = opsum.tile([128, QCH], f32, tag="o1", name="o1")
                    pend["tail"] = (qc - 1, po0, po1)
                    for kc in range(NK):
                        attn_iter(qc, kc, o0, o1)

                emit_pv(*pend["pv"])
                emit_proj(emit_tail(o0, o1), NQ - 1)

    nc.compile()
    return nc


def _round_fp32r(a):
    """Round to the compute dtype: fp32r (11-bit mantissa), fp16, or bf16."""
    if CDT == "float16":
        return np.ascontiguousarray(a).astype(np.float16)
    if CDT == "bfloat16":
        import ml_dtypes
        return np.ascontiguousarray(a).astype(ml_dtypes.bfloat16)
    b = np.ascontiguousarray(a).view(np.uint32)
    lsb = (b >> np.uint32(12)) & np.uint32(1)
    out = (b + np.uint32(0x7FF) + lsb) & np.uint32(0xFFFFF000)
    return out.view(np.float32)


def _cdt_np(a):
    if CDT == "float16":
        return a.astype(np.float16)
    if CDT == "bfloat16":
        import ml_dtypes
        return a.astype(ml_dtypes.bfloat16)
    return a


def _prep_inputs(x, W_qkv, b_qkv, W_proj):
    """Host-side shard prep. Returns per-core input maps for the SPMD kernel."""
    # [sc, p, dc, q] layout: xt[sc, p, dc, q] = x[sc*512+q, dc*128+p]
    xT = _round_fp32r(np.ascontiguousarray(
        x.reshape(NQ, QCH, NDC, 128).transpose(0, 3, 2, 1)))
    sel = np.zeros((128, 128), dtype=np.float32)
    sel[64, 0:64] = 1.0  # zsb partition 64 (recip Z0) -> bcast rows 0:64
    sel[32, 64:128] = 1.0  # zsb partition 32 (recip Z1) -> bcast rows 64:128
    sel = _cdt_np(sel)
    ident = _cdt_np(np.ascontiguousarray(np.vstack([np.eye(64, dtype=np.float32)] * 2)))
    patt = np.zeros(65, dtype=np.float32)
    patt[0] = 1.0   # vall col 64: ones column for head 0 sums
    patt[33] = 1.0  # vall col 97: ones column for head 1 sums (partition 32)
    vfill = np.ascontiguousarray(np.broadcast_to(patt, (128, NK, 65)))
    if EXP_BF16:
        import ml_dtypes
        vfill = vfill.astype(ml_dtypes.bfloat16)
    else:
        vfill = _cdt_np(vfill)
    zfill = _cdt_np(np.zeros((128, QCH), dtype=np.float32))
    efill = np.ones((128, 2 * QCH), dtype=np.float32)

    in_maps = []
    for c in range(NCORES):
        h0 = 2 * c
        idx = np.concatenate([
            np.arange(DIM + HD * h0, DIM + HD * h0 + 128),          # K rows
            np.arange(2 * DIM + HD * h0, 2 * DIM + HD * h0 + 128),  # V rows
            np.arange(HD * h0, HD * h0 + 128),                      # Q rows
        ])
        w_shard = W_qkv[idx]                                  # [384, 1024]
        # [p, dc, row]: wq[p, dc, r] = w_shard[r, dc*128+p]
        wqkvT = _round_fp32r(np.ascontiguousarray(
            w_shard.T.reshape(NDC, 128, 384).transpose(1, 0, 2)))
        bq = np.ascontiguousarray(b_qkv[idx].reshape(3, 128).T)  # [128, 3]
        wprojT = _round_fp32r(np.ascontiguousarray(W_proj[:, 128 * c:128 * (c + 1)].T))  # [128, 1024]
        in_maps.append({
            "xT": xT,
            "wqkvT": wqkvT,
            "bq": bq,
            "wprojT": wprojT,
            "sel": sel,
            "ident": ident,
            "vfill": vfill,
            "zfill": zfill,
        })
        if ABLATE == "noexp":
            in_maps[-1]["efill"] = efill
    return in_maps


EXP_BF16 = False
ABLATE = None
CDT = "float32r"


def _get_nc(loop_n=None):
    key = ("nc", loop_n, EXP_BF16, ABLATE, CDT)
    if key not in _COMPILED:
        _COMPILED[key] = _build_nc(
            loop_n, exp_bf16=EXP_BF16, ablate=ABLATE, cdt_name=CDT)
    return _COMPILED[key]


def run(x, W_qkv, b_qkv, W_proj, b_proj, trace=False, **trace_kwargs):
    """Run the sharded kernel; returns (y_full, BassKernelResults)."""
    from concourse.bass_utils import run_bass_kernel_spmd

    x = np.asarray(x, dtype=np.float32)
    W_qkv = np.asarray(W_qkv, dtype=np.float32)
    b_qkv = np.asarray(b_qkv, dtype=np.float32)
    W_proj = np.asarray(W_proj, dtype=np.float32)
    b_proj = np.asarray(b_proj, dtype=np.float32)

    nc = _get_nc()
    in_maps = _prep_inputs(x, W_qkv, b_qkv, W_proj)
    res = run_bass_kernel_spmd(
        nc, in_maps, core_ids=list(range(NCORES)), trace=trace, **trace_kwargs
    )
    y = np.zeros((SEQ, DIM), dtype=np.float32)
    for r in res.results:
        y += r["y"]
    y += b_proj
    return y, res


def kernel(x, W_qkv, b_qkv, W_proj, b_proj):
    y, _ = run(x, W_qkv, b_qkv, W_proj, b_proj, trace=False)
    return y



# revision 15
# speedup vs baseline: 7.0667x; 1.1439x over previous
"""Trainium2 Bass kernel for nn_Attention (dense transformer MHA forward).

Shapes: x [4096, 1024], 16 heads x head_dim 64, full softmax attention.

Sharding (8 cores, tensor-parallel over heads): each core owns 2 heads.
  - column-parallel qkv: core computes q,k,v for its 2 heads only
  - local attention for 2 heads
  - row-parallel proj: core computes a partial [4096, 1024] output
  - "all-reduce" = host-side sum of the 8 partials (+ b_proj once)

Device-kernel layout choices (per core, heads h0/h1):
  - everything is computed transposed: qkv^T [rows, seq] so that
    S^T = K^T.T-matmul works with seq-k on PSUM partitions and seq-q as the
    moving free dim; the softmax denominator is produced by the PE itself via
    a ones-column folded into the stationary V operand of the P@V matmul.
  - exp() runs on the Scalar engine straight out of PSUM (fused eviction),
    with the 1/sqrt(head_dim) folded into the activation's free scale.
    No max-subtraction: scores are ~N(0,1), exp is safe in fp32.
"""

import numpy as np

SEQ = 4096
DIM = 1024
HEADS = 16
HD = 64
NCORES = 8
QCH = 512          # q-chunk (matmul moving free dim)
KCH = 128          # k-chunk (contraction tile)
NQ = SEQ // QCH    # 8
NK = SEQ // KCH    # 32
NDC = DIM // 128   # 8 contraction chunks for the qkv projection

_COMPILED = {}


def _build_nc(loop_n=None, exp_bf16=False, ablate=None, cdt_name='float32r'):
    abl = set((ablate or '').split('+')) - {''}
    import concourse.bass as bass
    import concourse.bacc as bacc
    from concourse import mybir, tile

    f32 = mybir.dt.float32
    f32r = getattr(mybir.dt, cdt_name)  # compute dtype for all matmul operands
    bf16 = mybir.dt.bfloat16
    edt = bf16 if exp_bf16 else f32r
    nc = bacc.Bacc("TRN2", target_bir_lowering=False, debug=False)

    xT_d = nc.dram_tensor("xT", [NQ, 128, NDC, QCH], f32r, kind="ExternalInput")
    wqkvT_d = nc.dram_tensor("wqkvT", [128, NDC, 384], f32r, kind="ExternalInput")
    bq_d = nc.dram_tensor("bq", [128, 3], f32, kind="ExternalInput")
    wprojT_d = nc.dram_tensor("wprojT", [128, DIM], f32r, kind="ExternalInput")
    sel_d = nc.dram_tensor("sel", [128, 128], f32r, kind="ExternalInput")
    ident_d = nc.dram_tensor("ident", [128, 64], f32r, kind="ExternalInput")
    vfill_d = nc.dram_tensor("vfill", [128, NK, 65], edt, kind="ExternalInput")
    zfill_d = nc.dram_tensor("zfill", [128, QCH], f32r, kind="ExternalInput")
    efill_d = None
    if "noexp" in abl:
        efill_d = nc.dram_tensor("efill", [128, 2 * QCH], edt, kind="ExternalInput")
    y_d = nc.dram_tensor("y", [SEQ, DIM], f32, kind="ExternalOutput")

    EXP = mybir.ActivationFunctionType.Exp

    with tile.TileContext(nc) as tc, nc.allow_low_precision(
        reason="float32r (11-bit mantissa) matmul inputs, fp32 PSUM accumulate"
    ):
        with (
            tc.tile_pool(name="const", bufs=1) as const,
            tc.tile_pool(name="xpool", bufs=2) as xpool,
            tc.tile_pool(name="big", bufs=1) as big,
            tc.tile_pool(name="epool", bufs=3) as epool,
            tc.tile_pool(name="opool", bufs=2) as opool,
            tc.tile_pool(name="ypool", bufs=3) as ypool,
            tc.tile_pool(name="spsum", bufs=2, space="PSUM") as spsum,
            tc.tile_pool(name="opsum", bufs=1, space="PSUM") as opsum,
            tc.tile_pool(name="mpsum", bufs=2, space="PSUM") as mpsum,
        ):
            # ---- constants ----
            wq = const.tile([128, NDC, 384], f32r)
            nc.sync.dma_start(out=wq, in_=wqkvT_d.ap())
            wp = const.tile([128, DIM], f32r)
            nc.sync.dma_start(out=wp, in_=wprojT_d.ap())
            bq = const.tile([128, 3], f32)
            nc.sync.dma_start(out=bq, in_=bq_d.ap())
            sel = const.tile([128, 128], f32r)
            nc.sync.dma_start(out=sel, in_=sel_d.ap())
            idn = const.tile([128, 64], f32r)
            nc.sync.dma_start(out=idn, in_=ident_d.ap())
            e_const = None
            if "noexp" in abl:
                e_const = const.tile([128, 2 * QCH], edt, name="e_const")
                nc.sync.dma_start(out=e_const, in_=efill_d.ap())

            # ---- persistent SBUF state ----
            KT = big.tile([128, SEQ], f32r)   # rows 0:64 K^T h0, 64:128 K^T h1
            VT = big.tile([128, SEQ], f32r)
            QT = big.tile([128, SEQ], f32r)
            # per k-chunk stationary for P@V:
            #   cols 0:64 V_h0 | 64 ones | then h1 slab (65:193):
            #   local [0:32] zeros | [32] ones | [33:64] zeros | [64:128] V_h1
            #   so h1's Z lands on PSUM partition 32 (32-aligned APs only)
            vall = big.tile([128, NK, 193], edt)
            zsb = big.tile([128, QCH], f32r)  # softmax-recip staging rows 63/64

            # memset can't write f32r; DMA the static ones/zeros pattern in
            nc.sync.dma_start(out=zsb, in_=zfill_d.ap())
            nc.sync.dma_start(out=vall[:, :, 64:129], in_=vfill_d.ap())

            import contextlib
            loop_cm = (
                tc.For_i(0, loop_n, 1, hint_engines=(
                    mybir.EngineType.PE, mybir.EngineType.DVE,
                    mybir.EngineType.Activation, mybir.EngineType.SP,
                    mybir.EngineType.Pool,
                ))
                if loop_n else contextlib.nullcontext()
            )
            with loop_cm:
                # ---- qkv projection: qkv^T[row, s] for 384 rows = [K0 K1 V0 V1 Q0 Q1]
                xTr = xT_d.ap()  # host-tiled [sc, 128, dc, q] for contiguous DMA
                dests = [KT, VT, QT]

                def emit_qkv_sc(sc):
                    xs = xpool.tile([128, NDC, QCH], f32r, tag="xs", name="xs")
                    if "nodma" in abl:
                        # timing probe: 1/8 of input traffic, rest of xs stays garbage
                        nc.sync.dma_start(out=xs[:, 0:1, :], in_=xTr[sc, :, 0:1, :])
                    else:
                        nc.sync.dma_start(out=xs[:, 0:2, :], in_=xTr[sc, :, 0:2, :])
                        nc.sync.dma_start(out=xs[:, 2:5, :], in_=xTr[sc, :, 2:5, :])
                        nc.sync.dma_start(out=xs[:, 5:8, :], in_=xTr[sc, :, 5:8, :])
                    for m in range(3):
                        ps = mpsum.tile([128, QCH], f32, tag="mm", name="ps")
                        for dc in range(NDC):
                            nc.tensor.matmul(
                                ps,
                                lhsT=wq[:, dc, m * 128:(m + 1) * 128],
                                rhs=xs[:, dc, :],
                                start=(dc == 0),
                                stop=(dc == NDC - 1),
                            )
                        nc.vector.tensor_scalar_add(
                            dests[m][:, sc * QCH:(sc + 1) * QCH], ps, bq[:, m:m + 1]
                        )
                    # V^T -> V transposes for this s-chunk's 4 k-chunks
                    for kc in range(4 * sc, 4 * sc + 4):
                        for h in range(2):
                            tp = mpsum.tile([128, 64], f32r, tag="mm", name="tp")
                            nc.tensor.transpose(
                                tp, VT[64 * h:64 * h + 64, kc * 128:(kc + 1) * 128],
                                idn[64 * h:64 * h + 64, :]
                            )
                            dst = 0 if h == 0 else 129
                            nc.vector.tensor_copy(vall[:, kc, dst:dst + 64], tp)

                def emit_pv(e, kc, o0, o1):
                    nc.tensor.matmul(
                        o0[0:65, :], lhsT=vall[:, kc, 0:65], rhs=e[:, 0:QCH],
                        start=(kc == 0), stop=(kc == NK - 1),
                    )
                    if "mm2" not in abl:
                        nc.tensor.matmul(
                            o1, lhsT=vall[:, kc, 65:193], rhs=e[:, QCH:2 * QCH],
                            start=(kc == 0), stop=(kc == NK - 1),
                        )

                def emit_tail(o0, o1):
                    if "mm2" in abl:
                        o1 = o0  # timing probe only: h1 accumulator unused
                    # softmax denominators: Z0 at o0 row 64, Z1 at o1 row 32
                    nc.vector.reciprocal(zsb[64:65, :], o0[64:65, :])
                    nc.vector.reciprocal(zsb[32:33, :], o1[32:33, :])
                    zb = mpsum.tile([128, QCH], f32, tag="mm", name="zb")
                    nc.tensor.matmul(zb, lhsT=sel, rhs=zsb, start=True, stop=True)
                    zbs = opool.tile([128, QCH], f32, tag="zbs", name="zbs")
                    nc.vector.tensor_copy(zbs, zb)
                    ot = opool.tile([128, QCH], f32r, tag="ot", name="ot")
                    nc.vector.tensor_mul(ot[0:64, :], o0[0:64, :], zbs[0:64, :])
                    nc.vector.tensor_mul(ot[64:128, :], o1[64:128, :], zbs[64:128, :])
                    return ot

                def emit_proj(ot, qc):
                    for ss in range(4):
                        ysb = ypool.tile([128, DIM], f32, tag="y", name="ysb")
                        for oh in range(2):
                            yp = mpsum.tile([128, QCH], f32, tag="mm", name="yp")
                            nc.tensor.matmul(
                                yp,
                                lhsT=ot[:, ss * 128:(ss + 1) * 128],
                                rhs=wp[:, oh * QCH:(oh + 1) * QCH],
                                start=True, stop=True,
                            )
                            nc.vector.tensor_copy(ysb[:, oh * QCH:(oh + 1) * QCH], yp)
                        r0 = qc * QCH + ss * 128
                        if "nodma" not in abl:
                            nc.sync.dma_start(out=y_d.ap()[r0:r0 + 128, :], in_=ysb)

                # ---- attention, software-pipelined across engines ----
                # Per (qc, kc) iteration this emits, in PE program order:
                #   S-pair(qc,kc) -> [tail(qc-1) @ kc==1] -> PV(prev iter)
                #   -> [proj(qc-1) @ kc==3]
                # so the PE never sits behind the exp of its own iteration, and
                # the normalize/proj of the previous q-chunk hides inside this one.
                pend = {"pv": None, "tail": None, "proj": None}

                def attn_iter(qc, kc, o0, o1):
                    qsl = slice(qc * QCH, (qc + 1) * QCH)
                    ksl = slice(kc * 128, (kc + 1) * 128)
                    s_ps = spsum.tile([128, 2 * QCH], f32, tag="s", name="s_ps")
                    if "mm2" in abl:
                        nc.tensor.matmul(
                            s_ps[:, 0:QCH], lhsT=KT[:, ksl], rhs=QT[:, qsl],
                            start=True, stop=True,
                        )
                    else:
                        nc.tensor.matmul(
                            s_ps[:, 0:QCH], lhsT=KT[0:64, ksl], rhs=QT[0:64, qsl],
                            start=True, stop=True,
                        )
                        nc.tensor.matmul(
                            s_ps[:, QCH:2 * QCH], lhsT=KT[64:128, ksl],
                            rhs=QT[64:128, qsl],
                            start=True, stop=True,
                        )
                    if "noexp" in abl:
                        e = e_const
                    else:
                        e = epool.tile([128, 2 * QCH], edt, tag="e", name="e")
                        nc.scalar.activation(e, s_ps, EXP, scale=1.0 / np.sqrt(HD))
                    if kc == 1 and pend["tail"] is not None:
                        prev_qc, po0, po1 = pend["tail"]
                        pend["proj"] = (emit_tail(po0, po1), prev_qc)
                        pend["tail"] = None
                    if pend["pv"] is not None:
                        emit_pv(*pend["pv"])
                    pend["pv"] = (e, kc, o0, o1)
                    if kc == 3 and pend["proj"] is not None:
                        emit_proj(*pend["proj"])
                        pend["proj"] = None

                # qc 0 rides along with the qkv prologue: each s-chunk of qkv
                # unlocks 4 k-chunks of attention for q-chunk 0
                o0 = opsum.tile([128, QCH], f32, tag="o0", name="o0")
                o1 = opsum.tile([128, QCH], f32, tag="o1", name="o1")
                for sc in range(NQ):
                    emit_qkv_sc(sc)
                    for kc in range(4 * sc, 4 * sc + 4):
                        attn_iter(0, kc, o0, o1)

                for qc in range(1, NQ):
                    po0, po1 = o0, o1
                    o0 = opsum.tile([128, QCH], f32, tag="o0", name="o0")
                    o1 = opsum.tile([128, QCH], f32, tag="o1", name="o1")
                    pend["tail"] = (qc - 1, po0, po1)
                    for kc in range(NK):
                        attn_iter(qc, kc, o0, o1)

                emit_pv(*pend["pv"])
                emit_proj(emit_tail(o0, o1), NQ - 1)

    nc.compile()
    return nc


def _round_fp32r(a):
    """Round to the compute dtype: fp32r (11-bit mantissa), fp16, or bf16."""
    if CDT == "float16":
        return np.ascontiguousarray(a).astype(np.float16)
    if CDT == "bfloat16":
        import ml_dtypes
        return np.ascontiguousarray(a).astype(ml_dtypes.bfloat16)
    b = np.ascontiguousarray(a).view(np.uint32)
    lsb = (b >> np.uint32(12)) & np.uint32(1)
    out = (b + np.uint32(0x7FF) + lsb) & np.uint32(0xFFFFF000)
    return out.view(np.float32)


def _cdt_np(a):
    if CDT == "float16":
        return a.astype(np.float16)
    if CDT == "bfloat16":
        import ml_dtypes
        return a.astype(ml_dtypes.bfloat16)
    return a


def _prep_inputs(x, W_qkv, b_qkv, W_proj):
    """Host-side shard prep. Returns per-core input maps for the SPMD kernel."""
    # [sc, p, dc, q] layout: xt[sc, p, dc, q] = x[sc*512+q, dc*128+p]
    xT = _round_fp32r(np.ascontiguousarray(
        x.reshape(NQ, QCH, NDC, 128).transpose(0, 3, 2, 1)))
    sel = np.zeros((128, 128), dtype=np.float32)
    sel[64, 0:64] = 1.0  # zsb partition 64 (recip Z0) -> bcast rows 0:64
    sel[32, 64:128] = 1.0  # zsb partition 32 (recip Z1) -> bcast rows 64:128
    sel = _cdt_np(sel)
    ident = _cdt_np(np.ascontiguousarray(np.vstack([np.eye(64, dtype=np.float32)] * 2)))
    patt = np.zeros(65, dtype=np.float32)
    patt[0] = 1.0   # vall col 64: ones column for head 0 sums
    patt[33] = 1.0  # vall col 97: ones column for head 1 sums (partition 32)
    vfill = np.ascontiguousarray(np.broadcast_to(patt, (128, NK, 65)))
    if EXP_BF16:
        import ml_dtypes
        vfill = vfill.astype(ml_dtypes.bfloat16)
    else:
        vfill = _cdt_np(vfill)
    zfill = _cdt_np(np.zeros((128, QCH), dtype=np.float32))
    efill = np.ones((128, 2 * QCH), dtype=np.float32)

    in_maps = []
    for c in range(NCORES):
        h0 = 2 * c
        idx = np.concatenate([
            np.arange(DIM + HD * h0, DIM + HD * h0 + 128),          # K rows
            np.arange(2 * DIM + HD * h0, 2 * DIM + HD * h0 + 128),  # V rows
            np.arange(HD * h0, HD * h0 + 128),                      # Q rows
        ])
        w_shard = W_qkv[idx]                                  # [384, 1024]
        # [p, dc, row]: wq[p, dc, r] = w_shard[r, dc*128+p]
        wqkvT = _round_fp32r(np.ascontiguousarray(
            w_shard.T.reshape(NDC, 128, 384).transpose(1, 0, 2)))
        bq = np.ascontiguousarray(b_qkv[idx].reshape(3, 128).T)  # [128, 3]
        wprojT = _round_fp32r(np.ascontiguousarray(W_proj[:, 128 * c:128 * (c + 1)].T))  # [128, 1024]
        in_maps.append({
            "xT": xT,
            "wqkvT": wqkvT,
            "bq": bq,
            "wprojT": wprojT,
            "sel": sel,
            "ident": ident,
            "vfill": vfill,
            "zfill": zfill,
        })
        if ABLATE == "noexp":
            in_maps[-1]["efill"] = efill
    return in_maps


EXP_BF16 = False
ABLATE = None
CDT = "float32r"


def _get_nc(loop_n=None):
    key = ("nc", loop_n, EXP_BF16, ABLATE, CDT)
    if key not in _COMPILED:
        _COMPILED[key] = _build_nc(
            loop_n, exp_bf16=EXP_BF16, ablate=ABLATE, cdt_name=CDT)
    return _COMPILED[key]


def run(x, W_qkv, b_qkv, W_proj, b_proj, trace=False, **trace_kwargs):
    """Run the sharded kernel; returns (y_full, BassKernelResults)."""
    from concourse.bass_utils import run_bass_kernel_spmd

    x = np.asarray(x, dtype=np.float32)
    W_qkv = np.asarray(W_qkv, dtype=np.float32)
    b_qkv = np.asarray(b_qkv, dtype=np.float32)
    W_proj = np.asarray(W_proj, dtype=np.float32)
    b_proj = np.asarray(b_proj, dtype=np.float32)

    nc = _get_nc()
    in_maps = _prep_inputs(x, W_qkv, b_qkv, W_proj)
    res = run_bass_kernel_spmd(
        nc, in_maps, core_ids=list(range(NCORES)), trace=trace, **trace_kwargs
    )
    y = np.zeros((SEQ, DIM), dtype=np.float32)
    for r in res.results:
        y += r["y"]
    y += b_proj
    return y, res


def kernel(x, W_qkv, b_qkv, W_proj, b_proj):
    y, _ = run(x, W_qkv, b_qkv, W_proj, b_proj, trace=False)
    return y



# revision 17
# speedup vs baseline: 7.1042x; 1.0053x over previous
"""Trainium2 Bass kernel for nn_Attention (dense transformer MHA forward).

Shapes: x [4096, 1024], 16 heads x head_dim 64, full softmax attention.

Sharding (8 cores, tensor-parallel over heads): each core owns 2 heads.
  - column-parallel qkv: core computes q,k,v for its 2 heads only
  - local attention for 2 heads
  - row-parallel proj: core computes a partial [4096, 1024] output
  - "all-reduce" = host-side sum of the 8 partials (+ b_proj once)

Device-kernel layout choices (per core, heads h0/h1):
  - everything is computed transposed: qkv^T [rows, seq] so that
    S^T = K^T.T-matmul works with seq-k on PSUM partitions and seq-q as the
    moving free dim; the softmax denominator is produced by the PE itself via
    a ones-column folded into the stationary V operand of the P@V matmul.
  - exp() runs on the Scalar engine straight out of PSUM (fused eviction),
    with the 1/sqrt(head_dim) folded into the activation's free scale.
    No max-subtraction: scores are ~N(0,1), exp is safe in fp32.
"""

import numpy as np

SEQ = 4096
DIM = 1024
HEADS = 16
HD = 64
NCORES = 8
QCH = 512          # q-chunk (matmul moving free dim)
KCH = 128          # k-chunk (contraction tile)
NQ = SEQ // QCH    # 8
NK = SEQ // KCH    # 32
NDC = DIM // 128   # 8 contraction chunks for the qkv projection

_COMPILED = {}


def _build_nc(loop_n=None, exp_bf16=False, ablate=None, cdt_name='float32r'):
    abl = set((ablate or '').split('+')) - {''}
    import concourse.bass as bass
    import concourse.bacc as bacc
    from concourse import mybir, tile

    f32 = mybir.dt.float32
    f32r = getattr(mybir.dt, cdt_name)  # compute dtype for all matmul operands
    bf16 = mybir.dt.bfloat16
    edt = bf16 if exp_bf16 else f32r
    nc = bacc.Bacc("TRN2", target_bir_lowering=False, debug=False)

    xT_d = nc.dram_tensor("xT", [NQ, 128, NDC, QCH], f32r, kind="ExternalInput")
    wqkvT_d = nc.dram_tensor("wqkvT", [128, NDC, 384], f32r, kind="ExternalInput")
    bq_d = nc.dram_tensor("bq", [128, 3], f32, kind="ExternalInput")
    wprojT_d = nc.dram_tensor("wprojT", [128, DIM], f32r, kind="ExternalInput")
    sel_d = nc.dram_tensor("sel", [128, 128], f32r, kind="ExternalInput")
    ident_d = nc.dram_tensor("ident", [128, 64], f32r, kind="ExternalInput")
    vfill_d = nc.dram_tensor("vfill", [128, NK, 65], edt, kind="ExternalInput")
    zfill_d = nc.dram_tensor("zfill", [128, QCH], f32r, kind="ExternalInput")
    efill_d = None
    if "noexp" in abl:
        efill_d = nc.dram_tensor("efill", [128, 2 * QCH], edt, kind="ExternalInput")
    y_d = nc.dram_tensor("y", [SEQ, DIM], f32, kind="ExternalOutput")

    EXP = mybir.ActivationFunctionType.Exp

    with tile.TileContext(nc) as tc, nc.allow_low_precision(
        reason="float32r (11-bit mantissa) matmul inputs, fp32 PSUM accumulate"
    ):
        with (
            tc.tile_pool(name="const", bufs=1) as const,
            tc.tile_pool(name="xpool", bufs=2) as xpool,
            tc.tile_pool(name="big", bufs=1) as big,
            tc.tile_pool(name="epool", bufs=3) as epool,
            tc.tile_pool(name="opool", bufs=2) as opool,
            tc.tile_pool(name="ypool", bufs=3) as ypool,
            tc.tile_pool(name="spsum", bufs=2, space="PSUM") as spsum,
            tc.tile_pool(name="opsum", bufs=1, space="PSUM") as opsum,
            tc.tile_pool(name="mpsum", bufs=2, space="PSUM") as mpsum,
        ):
            # ---- constants ----
            wq = const.tile([128, NDC, 384], f32r)
            nc.sync.dma_start(out=wq, in_=wqkvT_d.ap())
            wp = const.tile([128, DIM], f32r)
            nc.sync.dma_start(out=wp, in_=wprojT_d.ap())
            bq = const.tile([128, 3], f32)
            nc.sync.dma_start(out=bq, in_=bq_d.ap())
            sel = const.tile([128, 128], f32r)
            nc.sync.dma_start(out=sel, in_=sel_d.ap())
            idn = const.tile([128, 64], f32r)
            nc.sync.dma_start(out=idn, in_=ident_d.ap())
            e_const = None
            if "noexp" in abl:
                e_const = const.tile([128, 2 * QCH], edt, name="e_const")
                nc.sync.dma_start(out=e_const, in_=efill_d.ap())

            # ---- persistent SBUF state ----
            KT = big.tile([128, SEQ], f32r)   # rows 0:64 K^T h0, 64:128 K^T h1
            VT = big.tile([128, SEQ], f32r)
            QT = big.tile([128, SEQ], f32r)
            # per k-chunk stationary for P@V:
            #   cols 0:64 V_h0 | 64 ones | then h1 slab (65:193):
            #   local [0:32] zeros | [32] ones | [33:64] zeros | [64:128] V_h1
            #   so h1's Z lands on PSUM partition 32 (32-aligned APs only)
            vall = big.tile([128, NK, 193], edt)
            zsb = big.tile([128, QCH], f32r)  # softmax-recip staging rows 63/64

            # memset can't write f32r; DMA the static ones/zeros pattern in
            nc.sync.dma_start(out=zsb, in_=zfill_d.ap())
            nc.sync.dma_start(out=vall[:, :, 64:129], in_=vfill_d.ap())

            import contextlib
            loop_cm = (
                tc.For_i(0, loop_n, 1, hint_engines=(
                    mybir.EngineType.PE, mybir.EngineType.DVE,
                    mybir.EngineType.Activation, mybir.EngineType.SP,
                    mybir.EngineType.Pool,
                ))
                if loop_n else contextlib.nullcontext()
            )
            with loop_cm:
                # ---- qkv projection: qkv^T[row, s] for 384 rows = [K0 K1 V0 V1 Q0 Q1]
                xTr = xT_d.ap()  # host-tiled [sc, 128, dc, q] for contiguous DMA
                dests = [KT, VT, QT]

                def emit_qkv_sc(sc):
                    xs = xpool.tile([128, NDC, QCH], f32r, tag="xs", name="xs")
                    if "nodma" in abl:
                        # timing probe: 1/8 of input traffic, rest of xs stays garbage
                        nc.sync.dma_start(out=xs[:, 0:1, :], in_=xTr[sc, :, 0:1, :])
                    else:
                        nc.sync.dma_start(out=xs[:, 0:2, :], in_=xTr[sc, :, 0:2, :])
                        nc.sync.dma_start(out=xs[:, 2:5, :], in_=xTr[sc, :, 2:5, :])
                        nc.sync.dma_start(out=xs[:, 5:8, :], in_=xTr[sc, :, 5:8, :])
                    for m in range(3):
                        ps = mpsum.tile([128, QCH], f32, tag="mm", name="ps")
                        for dc in range(NDC):
                            nc.tensor.matmul(
                                ps,
                                lhsT=wq[:, dc, m * 128:(m + 1) * 128],
                                rhs=xs[:, dc, :],
                                start=(dc == 0),
                                stop=(dc == NDC - 1),
                            )
                        nc.vector.tensor_scalar_add(
                            dests[m][:, sc * QCH:(sc + 1) * QCH], ps, bq[:, m:m + 1]
                        )
                    # V^T -> V transposes for this s-chunk's 4 k-chunks
                    for kc in range(4 * sc, 4 * sc + 4):
                        for h in range(2):
                            tp = mpsum.tile([128, 64], f32r, tag="mm", name="tp")
                            nc.tensor.transpose(
                                tp, VT[64 * h:64 * h + 64, kc * 128:(kc + 1) * 128],
                                idn[64 * h:64 * h + 64, :]
                            )
                            dst = 0 if h == 0 else 129
                            nc.vector.tensor_copy(vall[:, kc, dst:dst + 64], tp)

                def emit_pv(e, kc, o0, o1):
                    nc.tensor.matmul(
                        o0[0:65, :], lhsT=vall[:, kc, 0:65], rhs=e[:, 0:QCH],
                        start=(kc == 0), stop=(kc == NK - 1),
                    )
                    if "mm2" not in abl:
                        nc.tensor.matmul(
                            o1, lhsT=vall[:, kc, 65:193], rhs=e[:, QCH:2 * QCH],
                            start=(kc == 0), stop=(kc == NK - 1),
                        )

                def emit_tail(o0, o1):
                    if "mm2" in abl:
                        o1 = o0  # timing probe only: h1 accumulator unused
                    # softmax denominators: Z0 at o0 row 64, Z1 at o1 row 32
                    nc.vector.reciprocal(zsb[64:65, :], o0[64:65, :])
                    nc.vector.reciprocal(zsb[32:33, :], o1[32:33, :])
                    zb = mpsum.tile([128, QCH], f32, tag="mm", name="zb")
                    nc.tensor.matmul(zb, lhsT=sel, rhs=zsb, start=True, stop=True)
                    zbs = opool.tile([128, QCH], f32, tag="zbs", name="zbs")
                    nc.vector.tensor_copy(zbs, zb)
                    ot = opool.tile([128, QCH], f32r, tag="ot", name="ot")
                    nc.vector.tensor_mul(ot[0:64, :], o0[0:64, :], zbs[0:64, :])
                    nc.vector.tensor_mul(ot[64:128, :], o1[64:128, :], zbs[64:128, :])
                    return ot

                def emit_proj(ot, qc):
                    for ss in range(4):
                        ysb = ypool.tile([128, DIM], f32, tag="y", name="ysb")
                        for oh in range(2):
                            yp = mpsum.tile([128, QCH], f32, tag="mm", name="yp")
                            nc.tensor.matmul(
                                yp,
                                lhsT=ot[:, ss * 128:(ss + 1) * 128],
                                rhs=wp[:, oh * QCH:(oh + 1) * QCH],
                                start=True, stop=True,
                            )
                            nc.vector.tensor_copy(ysb[:, oh * QCH:(oh + 1) * QCH], yp)
                        r0 = qc * QCH + ss * 128
                        if "nodma" not in abl:
                            nc.sync.dma_start(out=y_d.ap()[r0:r0 + 128, :], in_=ysb)

                # ---- attention, software-pipelined across engines ----
                # Per (qc, kc) iteration this emits, in PE program order:
                #   S-pair(qc,kc) -> [tail(qc-1) @ kc==1] -> PV(prev iter)
                #   -> [proj(qc-1) @ kc==3]
                # so the PE never sits behind the exp of its own iteration, and
                # the normalize/proj of the previous q-chunk hides inside this one.
                pend = {"pv": None, "tail": None, "proj": None}

                def attn_iter(qc, kc, o0, o1):
                    qsl = slice(qc * QCH, (qc + 1) * QCH)
                    ksl = slice(kc * 128, (kc + 1) * 128)
                    s_ps = spsum.tile([128, 2 * QCH], f32, tag="s", name="s_ps")
                    if "mm2" in abl:
                        nc.tensor.matmul(
                            s_ps[:, 0:QCH], lhsT=KT[:, ksl], rhs=QT[:, qsl],
                            start=True, stop=True,
                        )
                    else:
                        nc.tensor.matmul(
                            s_ps[:, 0:QCH], lhsT=KT[0:64, ksl], rhs=QT[0:64, qsl],
                            start=True, stop=True,
                        )
                        nc.tensor.matmul(
                            s_ps[:, QCH:2 * QCH], lhsT=KT[64:128, ksl],
                            rhs=QT[64:128, qsl],
                            start=True, stop=True,
                        )
                    if "noexp" in abl:
                        e = e_const
                    else:
                        e = epool.tile([128, 2 * QCH], edt, tag="e", name="e")
                        nc.scalar.activation(e, s_ps, EXP, scale=1.0 / np.sqrt(HD))
                    if kc == 1 and pend["tail"] is not None:
                        prev_qc, po0, po1 = pend["tail"]
                        pend["proj"] = (emit_tail(po0, po1), prev_qc)
                        pend["tail"] = None
                    if pend["pv"] is not None:
                        emit_pv(*pend["pv"])
                    pend["pv"] = (e, kc, o0, o1)
                    if kc == 3 and pend["proj"] is not None:
                        emit_proj(*pend["proj"])
                        pend["proj"] = None

                # qc 0 rides along with the qkv prologue: each s-chunk of qkv
                # unlocks 4 k-chunks of attention for q-chunk 0
                o0 = opsum.tile([128, QCH], f32, tag="o0", name="o0")
                o1 = opsum.tile([128, QCH], f32, tag="o1", name="o1")
                for sc in range(NQ):
                    emit_qkv_sc(sc)
                    for kc in range(4 * sc, 4 * sc + 4):
                        attn_iter(0, kc, o0, o1)

                for qc in range(1, NQ):
                    po0, po1 = o0, o1
                    o0 = opsum.tile([128, QCH], f32, tag="o0", name="o0")
                    o1 = opsum.tile([128, QCH], f32, tag="o1", name="o1")
                    pend["tail"] = (qc - 1, po0, po1)
                    for kc in range(NK):
                        attn_iter(qc, kc, o0, o1)

                emit_pv(*pend["pv"])
                emit_proj(emit_tail(o0, o1), NQ - 1)

    nc.compile()
    return nc


def _round_fp32r(a):
    """Round to the compute dtype: fp32r (11-bit mantissa), fp16, or bf16."""
    if CDT == "float16":
        return np.ascontiguousarray(a).astype(np.float16)
    if CDT == "bfloat16":
        import ml_dtypes
        return np.ascontiguousarray(a).astype(ml_dtypes.bfloat16)
    b = np.ascontiguousarray(a).view(np.uint32)
    lsb = (b >> np.uint32(12)) & np.uint32(1)
    out = (b + np.uint32(0x7FF) + lsb) & np.uint32(0xFFFFF000)
    return out.view(np.float32)


def _cdt_np(a):
    if CDT == "float16":
        return a.astype(np.float16)
    if CDT == "bfloat16":
        import ml_dtypes
        return a.astype(ml_dtypes.bfloat16)
    return a


def _prep_inputs(x, W_qkv, b_qkv, W_proj):
    """Host-side shard prep. Returns per-core input maps for the SPMD kernel."""
    # [sc, p, dc, q] layout: xt[sc, p, dc, q] = x[sc*512+q, dc*128+p]
    xT = _round_fp32r(np.ascontiguousarray(
        x.reshape(NQ, QCH, NDC, 128).transpose(0, 3, 2, 1)))
    sel = np.zeros((128, 128), dtype=np.float32)
    sel[64, 0:64] = 1.0  # zsb partition 64 (recip Z0) -> bcast rows 0:64
    sel[32, 64:128] = 1.0  # zsb partition 32 (recip Z1) -> bcast rows 64:128
    sel = _cdt_np(sel)
    ident = _cdt_np(np.ascontiguousarray(np.vstack([np.eye(64, dtype=np.float32)] * 2)))
    patt = np.zeros(65, dtype=np.float32)
    patt[0] = 1.0   # vall col 64: ones column for head 0 sums
    patt[33] = 1.0  # vall col 97: ones column for head 1 sums (partition 32)
    vfill = np.ascontiguousarray(np.broadcast_to(patt, (128, NK, 65)))
    if EXP_BF16:
        import ml_dtypes
        vfill = vfill.astype(ml_dtypes.bfloat16)
    else:
        vfill = _cdt_np(vfill)
    zfill = _cdt_np(np.zeros((128, QCH), dtype=np.float32))
    efill = np.ones((128, 2 * QCH), dtype=np.float32)

    in_maps = []
    for c in range(NCORES):
        h0 = 2 * c
        idx = np.concatenate([
            np.arange(DIM + HD * h0, DIM + HD * h0 + 128),          # K rows
            np.arange(2 * DIM + HD * h0, 2 * DIM + HD * h0 + 128),  # V rows
            np.arange(HD * h0, HD * h0 + 128),                      # Q rows
        ])
        w_shard = W_qkv[idx]                                  # [384, 1024]
        # [p, dc, row]: wq[p, dc, r] = w_shard[r, dc*128+p]
        wqkvT = _round_fp32r(np.ascontiguousarray(
            w_shard.T.reshape(NDC, 128, 384).transpose(1, 0, 2)))
        bq = np.ascontiguousarray(b_qkv[idx].reshape(3, 128).T)  # [128, 3]
        wprojT = _round_fp32r(np.ascontiguousarray(W_proj[:, 128 * c:128 * (c + 1)].T))  # [128, 1024]
        in_maps.append({
            "xT": xT,
            "wqkvT": wqkvT,
            "bq": bq,
            "wprojT": wprojT,
            "sel": sel,
            "ident": ident,
            "vfill": vfill,
            "zfill": zfill,
        })
        if ABLATE == "noexp":
            in_maps[-1]["efill"] = efill
    return in_maps


EXP_BF16 = False
ABLATE = None
CDT = "float32r"


def _get_nc(loop_n=None):
    key = ("nc", loop_n, EXP_BF16, ABLATE, CDT)
    if key not in _COMPILED:
        _COMPILED[key] = _build_nc(
            loop_n, exp_bf16=EXP_BF16, ablate=ABLATE, cdt_name=CDT)
    return _COMPILED[key]


def run(x, W_qkv, b_qkv, W_proj, b_proj, trace=False, **trace_kwargs):
    """Run the sharded kernel; returns (y_full, BassKernelResults)."""
    from concourse.bass_utils import run_bass_kernel_spmd

    x = np.asarray(x, dtype=np.float32)
    W_qkv = np.asarray(W_qkv, dtype=np.float32)
    b_qkv = np.asarray(b_qkv, dtype=np.float32)
    W_proj = np.asarray(W_proj, dtype=np.float32)
    b_proj = np.asarray(b_proj, dtype=np.float32)

    nc = _get_nc()
    in_maps = _prep_inputs(x, W_qkv, b_qkv, W_proj)
    res = run_bass_kernel_spmd(
        nc, in_maps, core_ids=list(range(NCORES)), trace=trace, **trace_kwargs
    )
    y = np.zeros((SEQ, DIM), dtype=np.float32)
    for r in res.results:
        y += r["y"]
    y += b_proj
    return y, res


def kernel(x, W_qkv, b_qkv, W_proj, b_proj):
    y, _ = run(x, W_qkv, b_qkv, W_proj, b_proj, trace=False)
    return y

